# revision 17
# baseline (speedup 1.0000x reference)
"""Dempster-Shafer evidential module on 8 Trainium2 cores (v2).

Math (see numerics_sim.py; host-sim max rel err ~2e-3 vs fp32 reference):

  The per-step Dempster normalization cancels algebraically; the scan
  collapses to an affine recurrence per (batch b, class k):

      z_t = s_t*z_{t-1} + 2/3,   s_t = 1/3 + (u[k,t]/3)*rho[b,t]
      rho = si/(amax + 1e-4 - si),  si = exp(2g x.w - g|w|^2 + ln a - g|x|^2)
      y_k = z_T - 1;  out[b,k] = y_k/(sum_k y + 1);  out[b,C] = 1/(sum_k y + 1)

Key implementation points vs v1 (66.3us -> target ~24us):
  - -g|x|^2 is computed ON HOST in f64 and shipped as a [2, BL] hi/lo f32r
    row pair, injected into the T PSUM by ONE K=2 matmul pass. This removes
    the on-device x+xl add (17us Pool), the squares (8us ACT) and 4 matmul
    passes/quarter, and it was the dominant error source (1.5e-2 -> 2e-3).
  - ln(alpha)-g|w|^2 rides as the ACT Exp per-partition bias (no PE passes).
  - x.w keeps the exact 3-pass bf16 hi/lo form (wh@xh + wr@xh + wh@xl).
  - The +1/3 of s is added either by ONE K=2 f32r matmul pass (rows
    third_a/third_b sum exactly to fp32 1/3) or by the ACT PSUM->SBUF
    evacuation bias; segments are padded with 2 leading all-(1/3) columns
    which contract any cross-class state leak by 9x (no reset columns, so
    data1 is a plain 2/3 memset and all paths are uniform).
  - The 128-step Dempster fold is one tensor_tensor_scan per batch-chunk,
    split across DVE (8) and GpSimd/Pool (8) to balance engines.
  - x is DMAed in 4 quarter-packed transfers (xh+xl interleaved blocks,
    1024B descriptors); quarter 0 is split xh/xl so the PE starts earliest.
"""

import numpy as np

B, F, P, C = 16384, 512, 128, 10
NCORES = 8
BL = B // NCORES          # 2048 rows per core
NSUB = BL // 128          # 16 batch sub-tiles of 128
NQ = 4                    # column quarters (512 wide)
NPAD = 2                  # leading 1/3 pad cols per class segment
SEG = P + NPAD            # 130 columns per class segment
QN = C * SEG              # 1300 scan columns
OUTW = NSUB * (C + 1)     # 176 packed output columns
NSPLIT = [(0, 512), (512, 512), (1024, QN - 1024)]

# per (quarter, j) sub-tile: (third_add, scan_engine)
#   third_add: "pe" = K=2 f32r const pass into PSUM, scan reads PSUM
#              "act" = ACT Copy bias=1/3 PSUM->SBUF, scan reads SBUF
# NOTE: tensor_tensor_scan is DVE-only (neuronxcc rejects the scan opcode on
# Pool), so every scan runs on DVE and Pool takes dent/rho/amax instead.
MODES = [("pe", "dve"), ("act", "dve"), ("act", "dve"), ("act", "dve")]

_PROG = {}
REPS = 1


def _build_program():
    import concourse.bacc as bacc
    import concourse.bass as bass
    import concourse.tile as tile
    from concourse import bass_isa, mybir

    f32 = mybir.dt.float32
    bf16 = mybir.dt.bfloat16
    f32r = mybir.dt.float32r
    Alu = mybir.AluOpType
    Act = mybir.ActivationFunctionType

    nc = bacc.Bacc("TRN2", target_bir_lowering=False, debug=False)

    # DRAM inputs (see _host_prep for layouts)
    xq_d = nc.dram_tensor("xq", [128, 8 * NQ * 512], bf16, kind="ExternalInput").ap()
    whc_d = nc.dram_tensor("whc", [128, 1026], bf16, kind="ExternalInput").ap()
    ublk_d = nc.dram_tensor("ublk", [P, QN], f32r, kind="ExternalInput").ap()
    rows2_d = nc.dram_tensor("rows2", [2, QN + BL], f32r, kind="ExternalInput").ap()
    out_d = nc.dram_tensor("out", [128, OUTW], f32, kind="ExternalOutput").ap()

    with tile.TileContext(nc) as tc:
        for _rep in range(REPS):
            with (
                tc.tile_pool(name="const", bufs=1) as cpool,
                tc.tile_pool(name="xin", bufs=1) as xpool,
                tc.tile_pool(name="mid", bufs=1) as mpool,
                tc.tile_pool(name="scan", bufs=3) as spool,
                tc.tile_pool(name="pst", bufs=2, space=bass.MemorySpace.PSUM) as pst,
                tc.tile_pool(name="pq", bufs=2, space=bass.MemorySpace.PSUM) as pq,
            ):
                # ---- constants ----
                whc = cpool.tile([128, 1026], bf16, tag="whc")
                nc.sync.dma_start(whc[:], whc_d[:])
                wh = [whc[:, 128 * c:128 * (c + 1)] for c in range(4)]
                wr = [whc[:, 512 + 128 * c:512 + 128 * (c + 1)] for c in range(4)]
                crow = whc[:, 1024:1026].bitcast(f32)          # [128,1] per-proto bias

                # tiny rows2 (gx + 1/3 consts) right away: T's last pass and
                # the q_stage const pass depend on it
                rows2 = cpool.tile([2, QN + BL], f32r, tag="rows2")
                nc.sync.dma_start(rows2[:], rows2_d[:])
                crow2ab = rows2[:, 0:QN]
                gx = rows2[:, QN:QN + BL]

                xall = xpool.tile([128, 8 * NQ * 512], bf16, tag="xall")
                xv = xall[:].rearrange("p (b q n) -> p b q n", b=8, q=NQ)
                dv = xq_d[:].rearrange("p (b q n) -> p b q n", b=8, q=NQ)
                # quarter 0 split xh/xl for earliest PE start; q1-3 packed
                nc.sync.dma_start(xv[:, 0:4, 0, :], dv[:, 0:4, 0, :])
                nc.sync.dma_start(xv[:, 4:8, 0, :], dv[:, 4:8, 0, :])
                for q in range(1, NQ):
                    nc.sync.dma_start(xv[:, :, q, :], dv[:, :, q, :])

                # ublk lands last (first needed by q_stage(0) after T3);
                # split along NSPLIT so chunk 0 is ready earliest
                ublk = cpool.tile([P, QN], f32r, tag="ublk")
                for (o, n) in NSPLIT:
                    nc.sync.dma_start(ublk[:, o:o + n], ublk_d[:, o:o + n])

                ones2f = cpool.tile([2, 128], f32, tag="ones2")
                nc.gpsimd.memset(ones2f[:], 1.0)
                ones2 = ones2f[:].bitcast(f32r)          # [2,128] f32r AP
                data1 = cpool.tile([128, QN], f32, tag="data1")
                nc.gpsimd.memset(data1[:], 2.0 / 3.0)

                zf = mpool.tile([128, NSUB * C], f32, tag="zf")
                sz = mpool.tile([128, NSUB], f32, tag="sz")
                rho_q = []

                # PE p-state warmup: dummy matmuls on already-landed consts
                # start the 3us ramp clock while the x DMAs are in flight, so
                # T(0) runs at full speed instead of pstate_mid. Results are
                # discarded (T(0)'s start=True pass re-inits the PSUM bank).
                warm = pst.tile([128, 512], f32, name="Twarm", tag="T")
                for _ in range(10):
                    nc.tensor.matmul(warm[:], ones2, gx[:, 0:512],
                                     start=True, stop=True)

                # ---- per column-quarter q_stage (lags one quarter) ----
                def q_stage(q):
                    rho = rho_q[q]
                    for j in range(4):
                        m = 4 * q + j
                        third, seng = MODES[j]
                        qs = pq.tile([128, QN], f32, name=f"qs{m}", tag="qs")
                        for (o, n) in NSPLIT:
                            nc.tensor.matmul(
                                qs[:, o:o + n], rho[:, 128 * j:128 * (j + 1)],
                                ublk[:, o:o + n], start=True,
                                stop=(third != "pe"))
                            if third == "pe":
                                nc.tensor.matmul(
                                    qs[:, o:o + n], ones2, crow2ab[:, o:o + n],
                                    start=False, stop=True)
                        if third == "act":
                            src = spool.tile([128, QN], f32, name=f"sh{m}", tag="sh")
                            nc.scalar.activation(src[:], qs[:], Act.Copy, bias=1.0 / 3.0)
                        else:
                            src = qs
                        so = spool.tile([128, QN], f32, name=f"so{m}", tag="so")
                        eng = nc.vector if seng == "dve" else nc.gpsimd
                        eng.tensor_tensor_scan(
                            so[:], src[:], data1[:], initial=1.0,
                            op0=Alu.mult, op1=Alu.add)
                        sov = so[:].rearrange("p (k s) -> p k s", s=SEG)
                        # on DVE: follows its own scan in DVE.SEQ order, so no
                        # head-of-line blocking (on Pool it stalled the next
                        # quarter's sip/amax/dent/rho chain)
                        nc.vector.tensor_copy(
                            zf[:, C * m:C * (m + 1)], sov[:, :, SEG - 1])
                    # partial finals: z-sum for this quarter's 4 sub-tiles
                    nc.vector.tensor_reduce(
                        sz[:, 4 * q:4 * (q + 1)],
                        zf[:, C * 4 * q:C * 4 * (q + 1)].rearrange(
                            "p (s k) -> p s k", k=C),
                        axis=mybir.AxisListType.X, op=Alu.add)

                for q in range(NQ):
                    cs, ce = 512 * q, 512 * (q + 1)
                    T = pst.tile([128, 512], f32, name=f"T{q}", tag="T")
                    for c in range(4):
                        nc.tensor.matmul(T[:], wh[c], xv[:, c, q, :],
                                         start=(c == 0), stop=False)
                    for c in range(4):
                        nc.tensor.matmul(T[:], wr[c], xv[:, c, q, :],
                                         start=False, stop=False)
                    for c in range(4):
                        nc.tensor.matmul(T[:], wh[c], xv[:, 4 + c, q, :],
                                         start=False, stop=False)
                    nc.tensor.matmul(T[:], ones2, gx[:, cs:ce],
                                     start=False, stop=True)

                    si = mpool.tile([128, 512], f32, name=f"si{q}", tag="si", bufs=3)
                    nc.scalar.activation(si[:], T[:], Act.Exp, bias=crow)
                    # dent = max(si)+1e-4-si without TensorScalarPtr on Pool
                    # (that opcode is invalid on Pool in neuronxcc codegen):
                    # sip = si+eps (ACT, table-free Copy+bias, shortest chain
                    # hop since it follows Exp in ACT.SEQ order);
                    # amax = allreduce_max(sip); dent = amax-si (Pool)
                    sip = spool.tile([128, 512], f32, name=f"sip{q}", tag="sip")
                    nc.scalar.activation(sip[:], si[:], Act.Copy, bias=1e-4)
                    amax = mpool.tile([128, 512], f32, name=f"amax{q}", tag="amax", bufs=2)
                    nc.gpsimd.partition_all_reduce(
                        amax[:], sip[:], channels=128,
                        reduce_op=bass_isa.ReduceOp.max)
                    dent = spool.tile([128, 512], f32, name=f"dent{q}", tag="dent")
                    nc.gpsimd.tensor_sub(dent[:], amax[:], si[:])
                    # lag-2 software pipeline, issued BEFORE recip/rho: the
                    # scans' inputs are long ready, and recip(q) is only
                    # needed by q_stage(q) two iterations later — issuing it
                    # after the scans avoids DVE.SEQ head-of-line blocking.
                    if q >= 2:
                        q_stage(q - 2)
                    rec = mpool.tile([128, 512], f32, name=f"rec{q}", tag="rec", bufs=2)
                    nc.vector.reciprocal_approx_fast(rec[:], dent[:])
                    rho = mpool.tile([128, 512], f32r, name=f"rho{q}", tag="rho", bufs=4)
                    nc.gpsimd.tensor_mul(rho[:], si[:], rec[:])
                    rho_q.append(rho)
                q_stage(NQ - 2)
                q_stage(NQ - 1)

                # ---- finals: y = z-1, dr = 1/(sum_k z - 9), out = z*dr - dr
                # (per-quarter z-sums already accumulated into sz) ----
                nc.vector.tensor_scalar_add(sz[:], sz[:], -(C - 1.0))
                dr = mpool.tile([128, NSUB], f32, tag="dr")
                nc.vector.reciprocal(dr[:], sz[:])
                outsb = mpool.tile([128, OUTW], f32, tag="outsb")
                for s in range(NSUB):
                    nc.vector.tensor_scalar(
                        outsb[:, (C + 1) * s:(C + 1) * s + C],
                        zf[:, C * s:C * (s + 1)],
                        scalar1=dr[:, s:s + 1], scalar2=dr[:, s:s + 1],
                        op0=Alu.mult, op1=Alu.subtract)
                ov = outsb[:].rearrange("p (s k) -> p s k", k=C + 1)
                nc.vector.tensor_copy(ov[:, :, C], dr[:])
                nc.sync.dma_start(out_d[:], outsb[:])

    nc.compile()
    return nc


def _f32r_round(v):
    # float32r = RNE to 11 explicit mantissa bits (HW-verified).
    u = np.asarray(v, np.float32).view(np.uint32).astype(np.uint64)
    drop = 12
    half = np.uint64(1 << (drop - 1))
    odd = (u >> np.uint64(drop)) & np.uint64(1)
    u2 = (u + half - np.uint64(1) + odd) & np.uint64(~((1 << drop) - 1) & 0xFFFFFFFF)
    return u2.astype(np.uint32).view(np.float32)


def _host_prep(x, w, eta, xi, beta):
    """Host-side: shard/layout x, build tiny replicated param tensors."""
    import ml_dtypes

    x = np.asarray(x, np.float32)
    w = np.asarray(w, np.float32)
    eta = np.asarray(eta, np.float32).reshape(-1)
    xi = np.asarray(xi, np.float32).reshape(-1)
    beta = np.asarray(beta, np.float32)

    gamma = (eta.astype(np.float64)) ** 2                # [P]
    if np.ptp(gamma) != 0.0:
        raise NotImplementedError(
            "kernel assumes per-prototype-constant gamma (eta); the shipped "
            "problem uses eta = full(0.1)")
    alpha = 1.0 / (1.0 + np.exp(-xi.astype(np.float64)))
    wsq = (w.astype(np.float64) ** 2).sum(-1)            # [P]

    # whc: [128, 1026] bf16 = wh 4 c-blocks | wr 4 c-blocks | crow bitcast
    wt2g = (2.0 * gamma[None, :] * w.T.astype(np.float64)).astype(np.float32)  # [F,P]
    wh = wt2g.astype(ml_dtypes.bfloat16)
    wr = (wt2g.astype(np.float64) - wh.astype(np.float64)).astype(np.float32).astype(ml_dtypes.bfloat16)
    crow = (np.log(alpha) - gamma * wsq).astype(np.float32)  # [P]
    whc = np.zeros((128, 1026), ml_dtypes.bfloat16)
    whc[:, 0:512] = wh.reshape(4, 128, P).transpose(1, 0, 2).reshape(128, 512)
    whc[:, 512:1024] = wr.reshape(4, 128, P).transpose(1, 0, 2).reshape(128, 512)
    whc[:, 1024:1026] = crow.view(np.uint16).reshape(128, 2).view(ml_dtypes.bfloat16)

    # ublk: [P, QN] f32r scan coefficients (pads zero)
    b2 = beta.astype(np.float64) ** 2
    u = b2 / b2.sum(0, keepdims=True)                    # [C,P]
    ublk = np.zeros((P, QN), np.float32)
    for k in range(C):
        base = k * SEG + NPAD
        for t in range(P):
            v = u[k, t] * (1.0 if t == 0 else 1.0 / 3.0)
            ublk[t, base + t] = np.float32(v)

    # rows2: [2, QN + BL] = third_a/b const rows | -g|x|^2 hi/lo rows
    third_a = float(_f32r_round(np.float32(1.0 / 3.0)))
    third_b = np.float32(1.0 / 3.0 - third_a)
    rows2_const = np.zeros((2, QN), np.float32)
    rows2_const[0, :] = third_a
    rows2_const[1, :] = third_b

    shards = x.reshape(NCORES, BL, F)
    in_maps = []
    for i in range(NCORES):
        xs = shards[i]                                   # [BL, F]
        gxl = (-gamma[0]) * (xs.astype(np.float64) ** 2).sum(-1)  # [BL] f64
        ga = _f32r_round(gxl.astype(np.float32))
        gb = (gxl - ga.astype(np.float64)).astype(np.float32)
        rows2 = np.concatenate(
            [rows2_const, np.stack([ga, gb], axis=0)], axis=1)    # [2, QN+BL]

        xt = np.ascontiguousarray(xs.T)                  # [F, BL]
        xh = xt.astype(ml_dtypes.bfloat16)
        xl = (xt.astype(np.float64) - xh.astype(np.float64)).astype(np.float32).astype(ml_dtypes.bfloat16)
        # xq: [128, b(h*4+c), q, 512] -> [128, 8*NQ*512]
        xh4 = xh.reshape(4, 128, NQ, 512).transpose(1, 0, 2, 3)
        xl4 = xl.reshape(4, 128, NQ, 512).transpose(1, 0, 2, 3)
        xq = np.concatenate([xh4, xl4], axis=1).reshape(128, 8 * NQ * 512)

        in_maps.append({"xq": xq, "whc": whc, "ublk": ublk, "rows2": rows2})
    return in_maps


def _run(in_maps, trace=False):
    from concourse.bass_utils import run_bass_kernel_spmd

    if "nc" not in _PROG:
        _PROG["nc"] = _build_program()
    nc = _PROG["nc"]
    res = run_bass_kernel_spmd(
        nc, in_maps, core_ids=list(range(NCORES)), trace=trace)
    outs = []
    for i in range(NCORES):
        o = np.asarray(res.results[i]["out"])            # [128, 176]
        outs.append(o.reshape(128, NSUB, C + 1).transpose(1, 0, 2).reshape(BL, C + 1))
    full = np.concatenate(outs, axis=0).astype(np.float32)
    return full, res


def kernel(x, w, eta, xi, beta):
    in_maps = _host_prep(x, w, eta, xi, beta)
    full, _ = _run(in_maps, trace=False)
    return full
